# revision 1
# baseline (speedup 1.0000x reference)
"""BiLevelRoutingAttention Trainium2 kernel.

Sharding: data-parallel over (T*B)=8 cores; core = b*4 + t.
Host: windowize + transpose + region-routing top-k (0.005% of FLOPs).
Device: qkv projection (fp32), LIF spike bits, per-window gathered
kv/ksum contractions (bf16 bits, exact integer arithmetic), linear
attention with fused denominator column, output projection (fp32).
The top-k window indices (which depend only on batch b) are baked into
the program; cores select their variant via tc.If(partition_id).
"""

import os
import numpy as np

# problem constants (hardcoded per contract)
T, B, Lt, Lh, Lw, C = 4, 2, 8, 32, 32, 256
WT, WH, WW = 4, 4, 4
NW = WT * WH * WW              # 64 windows
PT, PH, PW = Lt // WT, Lh // WH, Lw // WW
WS = PT * PH * PW              # 128 tokens per window
H, HD = 8, C // 8
TOPK = 4
NTOK = NW * WS                 # 8192 tokens per (t,b) shard
N_CORES = 8

last_results = None            # stashed BassKernelResults for test harness
last_nc = None
last_in_maps = None


def _windowize(x):
    xw = x.reshape(T, B, WT, PT, WH, PH, WW, PW, C)
    xw = xw.transpose(0, 1, 2, 4, 6, 3, 5, 7, 8).reshape(T, B, NW, WS, C)
    return xw


def _unwindowize(ow):
    o = ow.reshape(T, B, WT, WH, WW, PT, PH, PW, C)
    o = o.transpose(0, 1, 2, 5, 3, 6, 4, 7, 8).reshape(T, B, Lt, Lh, Lw, C)
    return o


def _routing_idx(xw32):
    """Mimic reference routing in fp32: region scores -> top-4 window idx."""
    region = xw32.sum(0).mean(2)                           # [B,NW,C]
    scores = np.einsum('bic,bjc->bij', region, region) * np.float32(HD ** -0.5)
    # jax.lax.top_k tie-break = lowest index first; stable argsort matches
    idx = np.argsort(-scores, axis=-1, kind='stable')[:, :, :TOPK]
    return idx                                             # [B,NW,TOPK]


def _build_program(idx_by_b, debug=False):
    import concourse.bass as bass
    import concourse.mybir as mybir
    import concourse.tile as tile
    from concourse import bacc
    from concourse.masks import make_identity

    f32 = mybir.dt.float32
    f16 = mybir.dt.float16
    bf16 = mybir.dt.bfloat16

    nc = bacc.Bacc("TRN2", target_bir_lowering=False, debug=False,
                   num_devices=N_CORES)

    xwT = nc.dram_tensor("xwT", [C, NTOK], f32, kind="ExternalInput").ap()
    wq = nc.dram_tensor("wq", [C, 3 * C], f32, kind="ExternalInput").ap()
    bq = nc.dram_tensor("bq", [3 * C], f32, kind="ExternalInput").ap()
    wp = nc.dram_tensor("wp", [C, C], f32, kind="ExternalInput").ap()
    bp = nc.dram_tensor("bp", [C], f32, kind="ExternalInput").ap()
    masks = nc.dram_tensor("masks", [128, 528], f32, kind="ExternalInput").ap()
    out_d = nc.dram_tensor("out", [NTOK, C], f32, kind="ExternalOutput").ap()
    if debug:
        dbg_q = nc.dram_tensor("dbg_q", [128, NW * 256], mybir.dt.bfloat16, kind="ExternalOutput").ap()
        dbg_k = nc.dram_tensor("dbg_k", [128, NW * 256], mybir.dt.bfloat16, kind="ExternalOutput").ap()
        dbg_v = nc.dram_tensor("dbg_v", [128, NW * 260], mybir.dt.bfloat16, kind="ExternalOutput").ap()
        dbg_kvs = nc.dram_tensor("dbg_kvs", [64, 260], mybir.dt.float16, kind="ExternalOutput").ap()
        dbg_qTw = nc.dram_tensor("dbg_qTw", [64, 512], mybir.dt.float16, kind="ExternalOutput").ap()
        dbg_at = nc.dram_tensor("dbg_at", [128, 256], f32, kind="ExternalOutput").ap()
        dbg_dr = nc.dram_tensor("dbg_dr", [128, 8], f32, kind="ExternalOutput").ap()

    with tile.TileContext(nc) as tc:
        with (
            tc.tile_pool(name="const", bufs=1) as const_pool,
            tc.tile_pool(name="bits", bufs=1) as bits_pool,
            tc.tile_pool(name="xt", bufs=4) as xt_pool,
            tc.tile_pool(name="work", bufs=3) as work_pool,
            tc.tile_pool(name="tpsum", bufs=2, space="PSUM") as tpsum,
        ):
            # ---- resident constants ----
            wq_sb = const_pool.tile([128, 2 * 768], f32, tag="wq")
            for kc in range(2):
                nc.sync.dma_start(wq_sb[:, kc * 768:(kc + 1) * 768],
                                  wq[kc * 128:(kc + 1) * 128, :])
            wp_sb = const_pool.tile([128, 2 * 256], f32, tag="wp")
            for kc in range(2):
                nc.sync.dma_start(wp_sb[:, kc * 256:(kc + 1) * 256],
                                  wp[kc * 128:(kc + 1) * 128, :])
            ident_b = const_pool.tile([128, 128], bf16, tag="idb")
            make_identity(nc, ident_b)
            ident_f = const_pool.tile([128, 128], f32, tag="idf")
            make_identity(nc, ident_f)

            ones_row = const_pool.tile([1, 128], f32, tag="ones")
            nc.vector.memset(ones_row, 1.0)
            bq_row = const_pool.tile([1, 768], f32, tag="bqr")
            nc.sync.dma_start(bq_row, bq[None, :])
            bp_row = const_pool.tile([1, 256], f32, tag="bpr")
            nc.sync.dma_start(bp_row, bp[None, :])
            mask_sb = const_pool.tile([128, 528], f32, tag="masks")
            nc.sync.dma_start(mask_sb, masks)

            thr = const_pool.tile([128, 768], f32, tag="thr")
            bp_bc = const_pool.tile([128, 256], f32, tag="bpbc")

            # ---- bit tensors (resident) ----
            q_bits = bits_pool.tile([128, NW * 256], bf16, tag="qb")
            k_bits = bits_pool.tile([128, NW * 256], bf16, tag="kb")
            v_ext = bits_pool.tile([128, NW * 257], bf16, tag="vb")
            v_r = v_ext.rearrange("p (w d) -> p w d", d=257)
            nc.vector.memset(v_r[:, :, 256], 1.0)

            # ---- stage 1: qkv projection + LIF + q transpose ----
            with tc.tile_pool(name="qkv_ps", bufs=2, space="PSUM") as qkv_psum:
                # broadcast bias rows across partitions via ones-column matmul
                bc_ps = qkv_psum.tile([128, 768], f32, tag="qkv")
                nc.tensor.matmul(bc_ps[:, 0:512], ones_row, bq_row[:, 0:512],
                                 start=True, stop=True)
                nc.tensor.matmul(bc_ps[:, 512:768], ones_row,
                                 bq_row[:, 512:768], start=True, stop=True)
                # thr = 2 - b_qkv  (spike(x) fires iff qkv + b >= 2)
                nc.vector.tensor_scalar(out=thr[:, 0:512], in0=bc_ps[:, 0:512],
                                        scalar1=-1.0, scalar2=2.0,
                                        op0=mybir.AluOpType.mult,
                                        op1=mybir.AluOpType.add)
                nc.vector.tensor_scalar(out=thr[:, 512:768],
                                        in0=bc_ps[:, 512:768],
                                        scalar1=-1.0, scalar2=2.0,
                                        op0=mybir.AluOpType.mult,
                                        op1=mybir.AluOpType.add)
                bc_ps2 = qkv_psum.tile([128, 768], f32, tag="qkv")
                nc.tensor.matmul(bc_ps2[:, 0:256], ones_row, bp_row,
                                 start=True, stop=True)
                nc.scalar.copy(bp_bc, bc_ps2[:, 0:256])
                for n in range(NW):
                    xt0 = xt_pool.tile([128, 128], f32, tag="xt")
                    xt1 = xt_pool.tile([128, 128], f32, tag="xt")
                    nc.sync.dma_start(xt0, xwT[0:128, n * 128:(n + 1) * 128])
                    nc.sync.dma_start(xt1, xwT[128:256, n * 128:(n + 1) * 128])
                    ps = qkv_psum.tile([128, 768], f32, tag="qkv")
                    nc.tensor.matmul(ps[:, 0:512], xt0, wq_sb[:, 0:512],
                                     start=True, stop=False)
                    nc.tensor.matmul(ps[:, 0:512], xt1, wq_sb[:, 768:1280],
                                     start=False, stop=True)
                    nc.tensor.matmul(ps[:, 512:768], xt0, wq_sb[:, 512:768],
                                     start=True, stop=False)
                    nc.tensor.matmul(ps[:, 512:768], xt1, wq_sb[:, 1280:1536],
                                     start=False, stop=True)
                    # LIF spike bits: (qkv + b >= 2) == (matmul >= thr)
                    nc.vector.tensor_tensor(
                        out=q_bits[:, n * 256:(n + 1) * 256],
                        in0=ps[:, 0:256], in1=thr[:, 0:256],
                        op=mybir.AluOpType.is_ge)
                    nc.vector.tensor_tensor(
                        out=k_bits[:, n * 256:(n + 1) * 256],
                        in0=ps[:, 256:512], in1=thr[:, 256:512],
                        op=mybir.AluOpType.is_ge)
                    nc.vector.tensor_tensor(
                        out=v_r[:, n, 0:256],
                        in0=ps[:, 512:768], in1=thr[:, 512:768],
                        op=mybir.AluOpType.is_ge)

            # ---- stage 2: routed attention + projection ----
            def attention_stage(idx):
                with (
                    tc.tile_pool(name="kv_ps", bufs=2, space="PSUM") as kv_psum,
                    tc.tile_pool(name="at_ps", bufs=2, space="PSUM") as at_psum,
                    tc.tile_pool(name="pj_ps", bufs=2, space="PSUM") as pj_psum,
                ):
                    for n in range(NW):
                        kv0 = kv_psum.tile([128, 257], f32, tag="kv")
                        kv1 = kv_psum.tile([128, 257], f32, tag="kv")
                        js = [int(j) for j in idx[n]]
                        for jj, j in enumerate(js):
                            st, sp = jj == 0, jj == 3
                            nc.tensor.matmul(
                                kv0, k_bits[:, j * 256:j * 256 + 128],
                                v_ext[:, j * 257:(j + 1) * 257],
                                start=st, stop=sp)
                            nc.tensor.matmul(
                                kv1, k_bits[:, j * 256 + 128:(j + 1) * 256],
                                v_ext[:, j * 257:(j + 1) * 257],
                                start=st, stop=sp)
                        # masked copy -> block-diagonal kv + per-head ksum cols
                        kvs = work_pool.tile([128, 528], f16, tag="kvs")
                        for hf, kvh in enumerate([kv0, kv1]):
                            nc.vector.tensor_tensor(
                                out=kvs[:, hf * 264:hf * 264 + 256],
                                in0=kvh[:, 0:256],
                                in1=mask_sb[:, hf * 264:hf * 264 + 256],
                                op=mybir.AluOpType.mult)
                            nc.vector.tensor_tensor(
                                out=kvs[:, hf * 264 + 256:hf * 264 + 264],
                                in0=kvh[:, 256:257].to_broadcast([128, 8]),
                                in1=mask_sb[:, hf * 264 + 256:hf * 264 + 264],
                                op=mybir.AluOpType.mult)
                        # transpose q bits -> [c, s]
                        qT_w = work_pool.tile([128, 256], f16, tag="qTw")
                        for hf in range(2):
                            tp = tpsum.tile([128, 128], bf16, tag="tr")
                            nc.tensor.transpose(
                                tp,
                                q_bits[:, n * 256 + hf * 128:n * 256 + (hf + 1) * 128],
                                ident_b)
                            nc.scalar.copy(
                                qT_w[:, hf * 128:(hf + 1) * 128], tp)
                        # numerator + per-head D in one K=128 pair
                        ap_ = at_psum.tile([128, 264], f32, tag="at")
                        nc.tensor.matmul(ap_, qT_w[:, 0:128],
                                         kvs[:, 0:264], start=True, stop=False)
                        nc.tensor.matmul(ap_, qT_w[:, 128:256],
                                         kvs[:, 264:528], start=False, stop=True)
                        dr = work_pool.tile([128, 8], f32, tag="dr")
                        nc.vector.tensor_scalar_add(dr, ap_[:, 256:264], 1e-6)
                        nc.vector.reciprocal(dr, dr)
                        at = work_pool.tile([128, 256], f32, tag="attn")
                        for h in range(H):
                            nc.vector.tensor_scalar_mul(
                                at[:, h * 32:(h + 1) * 32],
                                ap_[:, h * 32:(h + 1) * 32],
                                dr[:, h:h + 1])
                        aT = work_pool.tile([128, 256], f32, tag="aT")
                        for kd in range(2):
                            tp = tpsum.tile([128, 128], f32, tag="tr")
                            nc.tensor.transpose(
                                tp, at[:, kd * 128:(kd + 1) * 128], ident_f)
                            nc.scalar.copy(aT[:, kd * 128:(kd + 1) * 128], tp)
                        pp = pj_psum.tile([128, 256], f32, tag="pj")
                        nc.tensor.matmul(pp, aT[:, 0:128], wp_sb[:, 0:256],
                                         start=True, stop=False)
                        nc.tensor.matmul(pp, aT[:, 128:256], wp_sb[:, 256:512],
                                         start=False, stop=True)
                        ob = work_pool.tile([128, 256], f32, tag="ob")
                        nc.vector.tensor_tensor(out=ob, in0=pp, in1=bp_bc,
                                                op=mybir.AluOpType.add)
                        nc.sync.dma_start(out_d[n * 128:(n + 1) * 128, :], ob)

            if debug:
                nc.sync.dma_start(dbg_q, q_bits)
                nc.sync.dma_start(dbg_k, k_bits0)
                nc.sync.dma_start(dbg_v, v_ext)
            pid = nc.partition_id()
            with tc.If(pid <= 3) as cmp:
                attention_stage(idx_by_b[0])
            with cmp.Else():
                attention_stage(idx_by_b[1])

    nc.compile()
    return nc


def kernel(x, W_qkv, b_qkv, W_proj, b_proj):
    global last_results
    from concourse import bass_utils

    x = np.asarray(x, dtype=np.float32)
    xw = _windowize(x)                                     # [T,B,NW,WS,C]
    idx = _routing_idx(xw)                                 # [B,NW,TOPK]

    nc = _build_program(idx)

    mask = np.zeros((128, 528), np.float32)
    for hf in range(2):
        for cr in range(128):
            h = hf * 4 + cr // 32                  # global head of row cr
            mask[cr, hf * 264 + h * 32:hf * 264 + (h + 1) * 32] = 1.0
            mask[cr, hf * 264 + 256 + h] = 1.0

    in_maps = []
    for core in range(N_CORES):
        b, t = divmod(core, T)
        xwT_c = np.ascontiguousarray(
            xw[t, b].reshape(NTOK, C).T)                   # [C, NTOK]
        in_maps.append({
            "xwT": xwT_c,
            "masks": mask,
            "wq": np.asarray(W_qkv, np.float32),
            "bq": np.asarray(b_qkv, np.float32),
            "wp": np.asarray(W_proj, np.float32),
            "bp": np.asarray(b_proj, np.float32),
        })

    res = bass_utils.run_bass_kernel_spmd(
        nc, in_maps, core_ids=list(range(N_CORES)), trace=False)
    last_results = res
    global last_nc, last_in_maps
    last_nc, last_in_maps = nc, in_maps

    ow = np.empty((T, B, NW, WS, C), np.float32)
    for core in range(N_CORES):
        b, t = divmod(core, T)
        ow[t, b] = res.results[core]["out"].reshape(NW, WS, C)
    return _unwindowize(ow)



# revision 5
# speedup vs baseline: 1.0034x; 1.0034x over previous
"""BiLevelRoutingAttention Trainium2 kernel (v2).

Sharding: data-parallel over (T*B)=8 cores; core = b*4 + t.
Host: windowize + transpose + region-routing top-k (0.005% of FLOPs).
Device, per core (8192 tokens, 64 windows of 128):
  stage 1: qkv projection in exact fp32 (spike bits are sensitive to
    <1e-6 perturbations near threshold); k,v computed token-major with
    the x-tile stationary, q computed directly TRANSPOSED (chan-major,
    Wq stationary) so no PE transposes are ever needed.
  stage 2 per window: routed kv as 8 half-width (N=129) bf16 matmuls
    accumulated over the topk windows (incl. a ones column -> ksum);
    masked block-diag kv + ksum-broadcast matrix feed a transposed
    numerator matmul producing [attn^T | D-replicated] in one PSUM
    bank; divide via max(D,1) + fast reciprocal (== ref within 1e-6);
    output projection straight from attn^T (bf16) + bias, DMA out.
The top-k window indices (which depend only on batch b) are baked into
the program; cores select their variant via tc.If(partition_id).
"""

import os
import numpy as np

# problem constants (hardcoded per contract)
T, B, Lt, Lh, Lw, C = 4, 2, 8, 32, 32, 256
WT, WH, WW = 4, 4, 4
NW = WT * WH * WW              # 64 windows
PT, PH, PW = Lt // WT, Lh // WH, Lw // WW
WS = PT * PH * PW              # 128 tokens per window
H, HD = 8, C // 8
TOPK = 4
NTOK = NW * WS                 # 8192 tokens per (t,b) shard
N_CORES = 8
NGRP = NW // 4                 # stage-1 token groups of 512

last_results = None            # stashed BassKernelResults for test harness
last_nc = None
last_in_maps = None


def _windowize(x):
    xw = x.reshape(T, B, WT, PT, WH, PH, WW, PW, C)
    xw = xw.transpose(0, 1, 2, 4, 6, 3, 5, 7, 8).reshape(T, B, NW, WS, C)
    return xw


def _unwindowize(ow):
    o = ow.reshape(T, B, WT, WH, WW, PT, PH, PW, C)
    o = o.transpose(0, 1, 2, 5, 3, 6, 4, 7, 8).reshape(T, B, Lt, Lh, Lw, C)
    return o


def _routing_idx(xw32):
    """Mimic reference routing in fp32: region scores -> top-4 window idx."""
    region = xw32.sum(0).mean(2)                           # [B,NW,C]
    scores = np.einsum('bic,bjc->bij', region, region) * np.float32(HD ** -0.5)
    # jax.lax.top_k tie-break = lowest index first; stable argsort matches
    idx = np.argsort(-scores, axis=-1, kind='stable')[:, :, :TOPK]
    return idx                                             # [B,NW,TOPK]


def _build_program(idx_by_b, single_branch=False):
    import concourse.bass as bass
    import concourse.mybir as mybir
    import concourse.tile as tile
    from concourse import bacc

    f32 = mybir.dt.float32
    bf16 = mybir.dt.bfloat16
    ge = mybir.AluOpType.is_ge
    mul = mybir.AluOpType.mult
    add = mybir.AluOpType.add

    nc = bacc.Bacc("TRN2", target_bir_lowering=False, debug=False,
                   num_devices=N_CORES)

    xwT = nc.dram_tensor("xwT", [C, NTOK], f32, kind="ExternalInput").ap()
    wq = nc.dram_tensor("wq", [C, 3 * C], f32, kind="ExternalInput").ap()
    bq = nc.dram_tensor("bq", [3 * C], f32, kind="ExternalInput").ap()
    wp = nc.dram_tensor("wp", [C, C], f32, kind="ExternalInput").ap()
    bp = nc.dram_tensor("bp", [C], f32, kind="ExternalInput").ap()
    masks = nc.dram_tensor("masks", [128, 128], f32, kind="ExternalInput").ap()
    out_d = nc.dram_tensor("out", [NTOK, C], f32, kind="ExternalOutput").ap()

    with tile.TileContext(nc) as tc:
        with (
            tc.tile_pool(name="const", bufs=1) as const_pool,
            tc.tile_pool(name="bits", bufs=1) as bits_pool,
            tc.tile_pool(name="xt", bufs=3) as xt_pool,
            tc.tile_pool(name="work", bufs=3) as work_pool,
        ):
            # ---- resident constants ----
            # k|v weight columns (wq cols 256:768), both c-chunks
            wkv_sb = const_pool.tile([128, 1024], f32, tag="wkv")
            for kc in range(2):
                nc.sync.dma_start(wkv_sb[:, kc * 512:(kc + 1) * 512],
                                  wq[kc * 128:(kc + 1) * 128, 256:768])
            # q weight columns (wq cols 0:256), both c-chunks — lhsT for qT
            wqq_sb = const_pool.tile([128, 512], f32, tag="wqq")
            for kc in range(2):
                nc.sync.dma_start(wqq_sb[:, kc * 256:(kc + 1) * 256],
                                  wq[kc * 128:(kc + 1) * 128, 0:256])
            wp32_sb = const_pool.tile([128, 512], f32, tag="wp32")
            for kc in range(2):
                nc.sync.dma_start(wp32_sb[:, kc * 256:(kc + 1) * 256],
                                  wp[kc * 128:(kc + 1) * 128, :])
            wpb_sb = const_pool.tile([128, 512], bf16, tag="wpb")
            nc.vector.tensor_copy(wpb_sb, wp32_sb)

            ones_row = const_pool.tile([1, 128], f32, tag="ones")
            nc.vector.memset(ones_row, 1.0)
            bq_row = const_pool.tile([1, 768], f32, tag="bqr")
            nc.sync.dma_start(bq_row, bq[None, :])
            bp_row = const_pool.tile([1, 256], f32, tag="bpr")
            nc.sync.dma_start(bp_row, bp[None, :])
            mask_sb = const_pool.tile([128, 128], f32, tag="masks")
            nc.sync.dma_start(mask_sb, masks)

            thr_kv = const_pool.tile([128, 512], f32, tag="thrkv")
            thrq = const_pool.tile([128, 2], f32, tag="thrq")
            bp_bc = const_pool.tile([128, 256], f32, tag="bpbc")

            # ---- bit tensors (resident) ----
            k_bits = bits_pool.tile([128, NW * 256], bf16, tag="kb")
            v_ext = bits_pool.tile([128, NW * 258], bf16, tag="vb")
            v_r = v_ext.rearrange("p (w x) -> p w x", x=258)
            nc.vector.memset(v_r[:, :, 128], 1.0)
            nc.vector.memset(v_r[:, :, 257], 1.0)
            qt0 = bits_pool.tile([128, NTOK], bf16, tag="qt0")
            qt1 = bits_pool.tile([128, NTOK], bf16, tag="qt1")
            qt = (qt0, qt1)

            # ---- init: broadcast bias rows via ones-column matmuls ----
            with tc.tile_pool(name="init_ps", bufs=1, space="PSUM") as ips:
                bc = ips.tile([128, 512], f32, tag="i0")
                nc.tensor.matmul(bc, ones_row, bq_row[:, 256:768],
                                 start=True, stop=True)
                # spike(x+b) fires iff matmul >= 2 - b
                nc.vector.tensor_scalar(out=thr_kv, in0=bc,
                                        scalar1=-1.0, scalar2=2.0,
                                        op0=mul, op1=add)
                bc2 = ips.tile([128, 512], f32, tag="i1")
                nc.tensor.matmul(bc2[:, 0:256], ones_row, bp_row,
                                 start=True, stop=True)
                nc.scalar.copy(bp_bc, bc2[:, 0:256])
                bc3 = ips.tile([128, 512], f32, tag="i2")
                for qd in range(2):
                    nc.tensor.matmul(bc3[:, qd:qd + 1],
                                     bq_row[:, qd * 128:(qd + 1) * 128],
                                     ones_row[:, 0:1], start=(qd == 0),
                                     stop=(qd == 1))
                nc.vector.tensor_scalar(out=thrq, in0=bc3[:, 0:2],
                                        scalar1=-1.0, scalar2=2.0,
                                        op0=mul, op1=add)

            # ---- stage 1: qkv projection (fp32) + LIF spike bits ----
            with (
                tc.tile_pool(name="s1kv_ps", bufs=2, space="PSUM") as s1kv,
                tc.tile_pool(name="s1q_ps", bufs=2, space="PSUM") as s1q,
            ):
                for g in range(NGRP):
                    xt_g = xt_pool.tile([128, 1024], f32, tag="xt")
                    for kc in range(2):
                        nc.sync.dma_start(
                            xt_g[:, kc * 512:(kc + 1) * 512],
                            xwT[kc * 128:(kc + 1) * 128,
                                g * 512:(g + 1) * 512])
                    for i in range(4):
                        w = 4 * g + i
                        ps = s1kv.tile([128, 512], f32, tag="kv")
                        nc.tensor.matmul(ps, xt_g[:, i * 128:(i + 1) * 128],
                                         wkv_sb[:, 0:512],
                                         start=True, stop=False)
                        nc.tensor.matmul(
                            ps, xt_g[:, 512 + i * 128:512 + (i + 1) * 128],
                            wkv_sb[:, 512:1024], start=False, stop=True)
                        nc.vector.tensor_tensor(
                            out=k_bits[:, w * 256:(w + 1) * 256],
                            in0=ps[:, 0:256], in1=thr_kv[:, 0:256], op=ge)
                        nc.vector.tensor_tensor(
                            out=v_r[:, w, 0:128],
                            in0=ps[:, 256:384], in1=thr_kv[:, 256:384], op=ge)
                        nc.vector.tensor_tensor(
                            out=v_r[:, w, 129:257],
                            in0=ps[:, 384:512], in1=thr_kv[:, 384:512], op=ge)
                    for qd in range(2):
                        qp = s1q.tile([128, 512], f32, tag="qt")
                        nc.tensor.matmul(qp,
                                         wqq_sb[:, qd * 128:(qd + 1) * 128],
                                         xt_g[:, 0:512],
                                         start=True, stop=False)
                        nc.tensor.matmul(
                            qp, wqq_sb[:, 256 + qd * 128:256 + (qd + 1) * 128],
                            xt_g[:, 512:1024], start=False, stop=True)
                        nc.vector.tensor_tensor(
                            out=qt[qd][:, g * 512:(g + 1) * 512],
                            in0=qp,
                            in1=thrq[:, qd:qd + 1].to_broadcast([128, 512]),
                            op=ge)

            # ---- stage 2: routed attention + projection ----
            def attention_stage(idx):
                order = sorted(range(NW),
                               key=lambda n: (max(int(max(idx[n])), n), n))
                with (
                    tc.tile_pool(name="kv2_ps", bufs=3, space="PSUM") as kv2,
                    tc.tile_pool(name="num_ps", bufs=3, space="PSUM") as nmp,
                    tc.tile_pool(name="pj_ps", bufs=2, space="PSUM") as pjp,
                ):
                    for n in order:
                        js = [int(j) for j in idx[n]]
                        kvp = kv2.tile([128, 512], f32, tag="kv")
                        for jj, j in enumerate(js):
                            nc.tensor.matmul(
                                kvp[:, 0:129],
                                k_bits[:, j * 256:j * 256 + 128],
                                v_r[:, j, 0:129],
                                start=(jj == 0), stop=False)
                            nc.tensor.matmul(
                                kvp[:, 129:258],
                                k_bits[:, j * 256 + 128:(j + 1) * 256],
                                v_r[:, j, 129:258],
                                start=False, stop=(jj == 3))
                        # masked block-diag kv + ksum-broadcast matrix (bf16,
                        # exact: counts << 256)
                        kvJ = work_pool.tile([128, 512], bf16, tag="kvJ")
                        nc.vector.tensor_tensor(
                            out=kvJ[:, 0:128], in0=kvp[:, 0:128],
                            in1=mask_sb, op=mul)
                        nc.vector.tensor_tensor(
                            out=kvJ[:, 128:256], in0=kvp[:, 129:257],
                            in1=mask_sb, op=mul)
                        nc.vector.tensor_tensor(
                            out=kvJ[:, 256:384],
                            in0=kvp[:, 128:129].to_broadcast([128, 128]),
                            in1=mask_sb, op=mul)
                        nc.vector.tensor_tensor(
                            out=kvJ[:, 384:512],
                            in0=kvp[:, 257:258].to_broadcast([128, 128]),
                            in1=mask_sb, op=mul)
                        # transposed numerator + replicated denominator:
                        # one PSUM bank, one accumulation group, 4 quarters
                        nump = nmp.tile([128, 512], f32, tag="num")
                        qs0 = qt0[:, n * 128:(n + 1) * 128]
                        qs1 = qt1[:, n * 128:(n + 1) * 128]
                        nc.tensor.matmul(nump[:, 0:128], kvJ[:, 0:128],
                                         qs0, start=True, stop=False)
                        nc.tensor.matmul(nump[:, 128:256], kvJ[:, 128:256],
                                         qs1, start=False, stop=False)
                        nc.tensor.matmul(nump[:, 256:384], kvJ[:, 256:384],
                                         qs0, start=False, stop=False)
                        nc.tensor.matmul(nump[:, 384:512], kvJ[:, 384:512],
                                         qs1, start=False, stop=True)
                        # attn = num / max(D,1)  (== num/(D+1e-6) within 1e-6;
                        # D==0 implies num==0)
                        nc.vector.tensor_scalar_max(
                            out=nump[:, 256:512], in0=nump[:, 256:512],
                            scalar1=1.0)
                        rec = work_pool.tile([128, 256], f32, tag="rec")
                        nc.vector.reciprocal_approx_fast(
                            out=rec, in_=nump[:, 256:512])
                        attnT = work_pool.tile([128, 256], bf16, tag="attnT")
                        nc.vector.tensor_tensor(
                            out=attnT, in0=nump[:, 0:256], in1=rec, op=mul)
                        # output projection from attn^T (bf16) + bias
                        pj = pjp.tile([128, 512], f32, tag="pj")
                        nc.tensor.matmul(pj[:, 0:256], attnT[:, 0:128],
                                         wpb_sb[:, 0:256],
                                         start=True, stop=False)
                        nc.tensor.matmul(pj[:, 0:256], attnT[:, 128:256],
                                         wpb_sb[:, 256:512],
                                         start=False, stop=True)
                        ob = work_pool.tile([128, 256], f32, tag="ob")
                        nc.vector.tensor_tensor(out=ob, in0=pj[:, 0:256],
                                                in1=bp_bc, op=add)
                        nc.sync.dma_start(out_d[n * 128:(n + 1) * 128, :], ob)

            if single_branch:
                attention_stage(idx_by_b[0])
            else:
                pid = nc.partition_id()
                with tc.If(pid <= 3) as cmp:
                    attention_stage(idx_by_b[0])
                with cmp.Else():
                    attention_stage(idx_by_b[1])

    nc.compile()
    return nc


def kernel(x, W_qkv, b_qkv, W_proj, b_proj):
    global last_results, last_nc, last_in_maps
    from concourse import bass_utils

    x = np.asarray(x, dtype=np.float32)
    xw = _windowize(x)                                     # [T,B,NW,WS,C]
    idx = _routing_idx(xw)                                 # [B,NW,TOPK]

    nc = _build_program(idx)

    # same-head block mask: mask[d, e] = (d//32 == e//32)
    r = np.arange(128) // 32
    mask = (r[:, None] == r[None, :]).astype(np.float32)

    in_maps = []
    for core in range(N_CORES):
        b, t = divmod(core, T)
        xwT_c = np.ascontiguousarray(
            xw[t, b].reshape(NTOK, C).T)                   # [C, NTOK]
        in_maps.append({
            "xwT": xwT_c,
            "masks": mask,
            "wq": np.asarray(W_qkv, np.float32),
            "bq": np.asarray(b_qkv, np.float32),
            "wp": np.asarray(W_proj, np.float32),
            "bp": np.asarray(b_proj, np.float32),
        })

    res = bass_utils.run_bass_kernel_spmd(
        nc, in_maps, core_ids=list(range(N_CORES)), trace=False)
    last_results = res
    last_nc, last_in_maps = nc, in_maps

    ow = np.empty((T, B, NW, WS, C), np.float32)
    for core in range(N_CORES):
        b, t = divmod(core, T)
        ow[t, b] = res.results[core]["out"].reshape(NW, WS, C)
    return _unwindowize(ow)


# revision 13
# speedup vs baseline: 1.1895x; 1.1855x over previous
"""BiLevelRoutingAttention Trainium2 kernel (v2).

Sharding: data-parallel over (T*B)=8 cores; core = b*4 + t.
Host: windowize + transpose + region-routing top-k (0.005% of FLOPs).
Device, per core (8192 tokens, 64 windows of 128):
  stage 1: qkv projection in exact fp32 (spike bits are sensitive to
    <1e-6 perturbations near threshold); k,v computed token-major with
    the x-tile stationary, q computed directly TRANSPOSED (chan-major,
    Wq stationary) so no PE transposes are ever needed.
  stage 2 per window: routed kv as 8 half-width (N=129) bf16 matmuls
    accumulated over the topk windows (incl. a ones column -> ksum);
    masked block-diag kv + ksum-broadcast matrix feed a transposed
    numerator matmul producing [attn^T | D-replicated] in one PSUM
    bank; divide via max(D,1) + fast reciprocal (== ref within 1e-6);
    output projection straight from attn^T (bf16) + bias, DMA out.
The top-k window indices (which depend only on batch b) are baked into
the program; cores select their variant via tc.If(partition_id).
"""

import os
import numpy as np

# problem constants (hardcoded per contract)
T, B, Lt, Lh, Lw, C = 4, 2, 8, 32, 32, 256
WT, WH, WW = 4, 4, 4
NW = WT * WH * WW              # 64 windows
PT, PH, PW = Lt // WT, Lh // WH, Lw // WW
WS = PT * PH * PW              # 128 tokens per window
H, HD = 8, C // 8
TOPK = 4
NTOK = NW * WS                 # 8192 tokens per (t,b) shard
N_CORES = 8
NGRP = NW // 4                 # stage-1 token groups of 512

last_results = None            # stashed BassKernelResults for test harness
last_nc = None
last_in_maps = None


def _windowize(x):
    xw = x.reshape(T, B, WT, PT, WH, PH, WW, PW, C)
    xw = xw.transpose(0, 1, 2, 4, 6, 3, 5, 7, 8).reshape(T, B, NW, WS, C)
    return xw


def _unwindowize(ow):
    o = ow.reshape(T, B, WT, WH, WW, PT, PH, PW, C)
    o = o.transpose(0, 1, 2, 5, 3, 6, 4, 7, 8).reshape(T, B, Lt, Lh, Lw, C)
    return o


def _routing_idx(xw32):
    """Mimic reference routing in fp32: region scores -> top-4 window idx."""
    region = xw32.sum(0).mean(2)                           # [B,NW,C]
    scores = np.einsum('bic,bjc->bij', region, region) * np.float32(HD ** -0.5)
    # jax.lax.top_k tie-break = lowest index first; stable argsort matches
    idx = np.argsort(-scores, axis=-1, kind='stable')[:, :, :TOPK]
    return idx                                             # [B,NW,TOPK]


def _build_program(idx_by_b, single_branch=False):
    import concourse.bass as bass
    import concourse.mybir as mybir
    import concourse.tile as tile
    from concourse import bacc

    f32 = mybir.dt.float32
    bf16 = mybir.dt.bfloat16
    ge = mybir.AluOpType.is_ge
    mul = mybir.AluOpType.mult
    add = mybir.AluOpType.add

    nc = bacc.Bacc("TRN2", target_bir_lowering=False, debug=False,
                   num_devices=N_CORES)

    f16 = mybir.dt.float16

    xwT = nc.dram_tensor("xwT", [C, NTOK], f32, kind="ExternalInput").ap()
    wq = nc.dram_tensor("wq", [C, 3 * C], f32, kind="ExternalInput").ap()
    bq = nc.dram_tensor("bq", [3 * C], f32, kind="ExternalInput").ap()
    wp = nc.dram_tensor("wp", [C, C], f32, kind="ExternalInput").ap()
    bp = nc.dram_tensor("bp", [C], f32, kind="ExternalInput").ap()
    masks = nc.dram_tensor("masks", [128, 128], f32, kind="ExternalInput").ap()
    out_d = nc.dram_tensor("out", [NTOK, C], f16, kind="ExternalOutput").ap()

    with tile.TileContext(nc) as tc:
        with (
            tc.tile_pool(name="const", bufs=1) as const_pool,
            tc.tile_pool(name="bits", bufs=1) as bits_pool,
            tc.tile_pool(name="work", bufs=3) as work_pool,
        ):
            # ---- resident constants ----
            # k|v weight columns (wq cols 256:768), both c-chunks
            wkv_sb = const_pool.tile([128, 1024], f32, tag="wkv")
            for kc in range(2):
                nc.sync.dma_start(wkv_sb[:, kc * 512:(kc + 1) * 512],
                                  wq[kc * 128:(kc + 1) * 128, 256:768])
            # q weight columns (wq cols 0:256), both c-chunks — lhsT for qT
            wqq_sb = const_pool.tile([128, 512], f32, tag="wqq")
            for kc in range(2):
                nc.sync.dma_start(wqq_sb[:, kc * 256:(kc + 1) * 256],
                                  wq[kc * 128:(kc + 1) * 128, 0:256])
            wp32_sb = const_pool.tile([128, 512], f32, tag="wp32")
            for kc in range(2):
                nc.sync.dma_start(wp32_sb[:, kc * 256:(kc + 1) * 256],
                                  wp[kc * 128:(kc + 1) * 128, :])
            wpb_sb = const_pool.tile([128, 512], bf16, tag="wpb")
            nc.vector.tensor_copy(wpb_sb, wp32_sb)

            ones_row = const_pool.tile([1, 128], f32, tag="ones")
            nc.vector.memset(ones_row, 1.0)
            bq_row = const_pool.tile([1, 768], f32, tag="bqr")
            nc.sync.dma_start(bq_row, bq[None, :])
            bp_row = const_pool.tile([1, 256], f32, tag="bpr")
            nc.sync.dma_start(bp_row, bp[None, :])
            mask_sb = const_pool.tile([128, 128], f32, tag="masks")
            nc.sync.dma_start(mask_sb, masks)

            thr_kv = const_pool.tile([128, 512], f32, tag="thrkv")
            thrq = const_pool.tile([128, 2], f32, tag="thrq")
            bp_bc = const_pool.tile([128, 256], f32, tag="bpbc")

            # ---- bit tensors (resident) ----
            k_bits = bits_pool.tile([128, NW * 256], bf16, tag="kb")
            v_ext = bits_pool.tile([128, NW * 258], bf16, tag="vb")
            v_r = v_ext.rearrange("p (w x) -> p w x", x=258)
            nc.vector.memset(v_r[:, :, 128], 1.0)
            nc.vector.memset(v_r[:, :, 257], 1.0)
            qt0 = bits_pool.tile([128, NTOK], bf16, tag="qt0")
            qt1 = bits_pool.tile([128, NTOK], bf16, tag="qt1")
            qt = (qt0, qt1)

            # resident x^T [c-chunk, tokens]: 4 big DMAs split across the
            # two HWDGE queues (SP, ACT) — per-queue DMA bw is the kernel
            # bottleneck, large transfers on both queues maximize it
            x_res = bits_pool.tile([128, 2 * NTOK], f32, tag="xres")
            half = NTOK // 2
            for kc in range(2):
                for hf in range(2):
                    eng = nc.sync if kc == 0 else nc.scalar
                    eng.dma_start(
                        x_res[:, kc * NTOK + hf * half:
                              kc * NTOK + (hf + 1) * half],
                        xwT[kc * 128:(kc + 1) * 128,
                            hf * half:(hf + 1) * half])

            # ---- init: broadcast bias rows via ones-column matmuls ----
            with tc.tile_pool(name="init_ps", bufs=1, space="PSUM") as ips:
                bc = ips.tile([128, 512], f32, tag="i0")
                nc.tensor.matmul(bc, ones_row, bq_row[:, 256:768],
                                 start=True, stop=True)
                # spike(x+b) fires iff matmul >= 2 - b
                nc.vector.tensor_scalar(out=thr_kv, in0=bc,
                                        scalar1=-1.0, scalar2=2.0,
                                        op0=mul, op1=add)
                bc2 = ips.tile([128, 512], f32, tag="i1")
                nc.tensor.matmul(bc2[:, 0:256], ones_row, bp_row,
                                 start=True, stop=True)
                nc.scalar.copy(bp_bc, bc2[:, 0:256])
                bc3 = ips.tile([128, 512], f32, tag="i2")
                for qd in range(2):
                    nc.tensor.matmul(bc3[:, qd:qd + 1],
                                     bq_row[:, qd * 128:(qd + 1) * 128],
                                     ones_row[:, 0:1], start=(qd == 0),
                                     stop=(qd == 1))
                nc.vector.tensor_scalar(out=thrq, in0=bc3[:, 0:2],
                                        scalar1=-1.0, scalar2=2.0,
                                        op0=mul, op1=add)

            # ---- stage 1: qkv projection (fp32) + LIF spike bits ----
            with (
                tc.tile_pool(name="s1kv_ps", bufs=2, space="PSUM") as s1kv,
                tc.tile_pool(name="s1q_ps", bufs=2, space="PSUM") as s1q,
            ):
                for g in range(NGRP):
                    xc0 = x_res[:, g * 512:(g + 1) * 512]
                    xc1 = x_res[:, NTOK + g * 512:NTOK + (g + 1) * 512]
                    for i in range(4):
                        w = 4 * g + i
                        ps = s1kv.tile([128, 512], f32, tag="kv")
                        nc.tensor.matmul(ps, xc0[:, i * 128:(i + 1) * 128],
                                         wkv_sb[:, 0:512],
                                         start=True, stop=False)
                        nc.tensor.matmul(
                            ps, xc1[:, i * 128:(i + 1) * 128],
                            wkv_sb[:, 512:1024], start=False, stop=True)
                        nc.vector.tensor_tensor(
                            out=k_bits[:, w * 256:(w + 1) * 256],
                            in0=ps[:, 0:256], in1=thr_kv[:, 0:256], op=ge)
                        nc.vector.tensor_tensor(
                            out=v_r[:, w, 0:128],
                            in0=ps[:, 256:384], in1=thr_kv[:, 256:384], op=ge)
                        nc.vector.tensor_tensor(
                            out=v_r[:, w, 129:257],
                            in0=ps[:, 384:512], in1=thr_kv[:, 384:512], op=ge)
                    for qd in range(2):
                        qp = s1q.tile([128, 512], f32, tag="qt")
                        nc.tensor.matmul(qp,
                                         wqq_sb[:, qd * 128:(qd + 1) * 128],
                                         xc0, start=True, stop=False)
                        nc.tensor.matmul(
                            qp, wqq_sb[:, 256 + qd * 128:256 + (qd + 1) * 128],
                            xc1, start=False, stop=True)
                        nc.vector.tensor_tensor(
                            out=qt[qd][:, g * 512:(g + 1) * 512],
                            in0=qp,
                            in1=thrq[:, qd:qd + 1].to_broadcast([128, 512]),
                            op=ge)

            # ---- stage 2: routed attention + projection ----
            def attention_stage(idx):
                order = sorted(range(NW),
                               key=lambda n: (max(int(max(idx[n])), n), n))
                with (
                    tc.tile_pool(name="kv2_ps", bufs=3, space="PSUM") as kv2,
                    tc.tile_pool(name="num_ps", bufs=3, space="PSUM") as nmp,
                    tc.tile_pool(name="pj_ps", bufs=2, space="PSUM") as pjp,
                ):
                    for wi, n in enumerate(order):
                        js = [int(j) for j in idx[n]]
                        kvp = kv2.tile([128, 512], f32, tag="kv")
                        for jj, j in enumerate(js):
                            nc.tensor.matmul(
                                kvp[:, 0:129],
                                k_bits[:, j * 256:j * 256 + 128],
                                v_r[:, j, 0:129],
                                start=(jj == 0), stop=False)
                            nc.tensor.matmul(
                                kvp[:, 129:258],
                                k_bits[:, j * 256 + 128:(j + 1) * 256],
                                v_r[:, j, 129:258],
                                start=False, stop=(jj == 3))
                        # masked block-diag kv + ksum-broadcast matrix (bf16,
                        # exact: counts << 256)
                        kvJ = work_pool.tile([128, 512], bf16, tag="kvJ")
                        nc.vector.tensor_tensor(
                            out=kvJ[:, 0:128], in0=kvp[:, 0:128],
                            in1=mask_sb, op=mul)
                        nc.vector.tensor_tensor(
                            out=kvJ[:, 128:256], in0=kvp[:, 129:257],
                            in1=mask_sb, op=mul)
                        nc.vector.tensor_tensor(
                            out=kvJ[:, 256:384],
                            in0=kvp[:, 128:129].to_broadcast([128, 128]),
                            in1=mask_sb, op=mul)
                        nc.vector.tensor_tensor(
                            out=kvJ[:, 384:512],
                            in0=kvp[:, 257:258].to_broadcast([128, 128]),
                            in1=mask_sb, op=mul)
                        # transposed numerator + replicated denominator:
                        # one PSUM bank, one accumulation group, 4 quarters
                        nump = nmp.tile([128, 512], f32, tag="num")
                        qs0 = qt0[:, n * 128:(n + 1) * 128]
                        qs1 = qt1[:, n * 128:(n + 1) * 128]
                        nc.tensor.matmul(nump[:, 0:128], kvJ[:, 0:128],
                                         qs0, start=True, stop=False)
                        nc.tensor.matmul(nump[:, 128:256], kvJ[:, 128:256],
                                         qs1, start=False, stop=False)
                        nc.tensor.matmul(nump[:, 256:384], kvJ[:, 256:384],
                                         qs0, start=False, stop=False)
                        nc.tensor.matmul(nump[:, 384:512], kvJ[:, 384:512],
                                         qs1, start=False, stop=True)
                        # attn = num / max(D,1)  (== num/(D+1e-6) within 1e-6;
                        # D==0 implies num==0)
                        nc.vector.tensor_scalar_max(
                            out=nump[:, 256:512], in0=nump[:, 256:512],
                            scalar1=1.0)
                        rec = work_pool.tile([128, 256], f32, tag="rec")
                        nc.vector.reciprocal_approx_fast(
                            out=rec, in_=nump[:, 256:512])
                        attnT = work_pool.tile([128, 256], bf16, tag="attnT")
                        nc.vector.tensor_tensor(
                            out=attnT, in0=nump[:, 0:256], in1=rec, op=mul)
                        # output projection from attn^T (bf16) + bias
                        pj = pjp.tile([128, 512], f32, tag="pj")
                        nc.tensor.matmul(pj[:, 0:256], attnT[:, 0:128],
                                         wpb_sb[:, 0:256],
                                         start=True, stop=False)
                        nc.tensor.matmul(pj[:, 0:256], attnT[:, 128:256],
                                         wpb_sb[:, 256:512],
                                         start=False, stop=True)
                        ob = work_pool.tile([128, 256], f16, tag="ob")
                        nc.vector.tensor_tensor(out=ob, in0=pj[:, 0:256],
                                                in1=bp_bc, op=add)
                        eng = nc.sync if wi % 2 == 0 else nc.scalar
                        eng.dma_start(out_d[n * 128:(n + 1) * 128, :], ob)

            if single_branch:
                attention_stage(idx_by_b[0])
            else:
                pid = nc.partition_id()
                with tc.If(pid <= 3) as cmp:
                    attention_stage(idx_by_b[0])
                with cmp.Else():
                    attention_stage(idx_by_b[1])

    nc.compile()
    return nc


def kernel(x, W_qkv, b_qkv, W_proj, b_proj):
    global last_results, last_nc, last_in_maps
    from concourse import bass_utils

    x = np.asarray(x, dtype=np.float32)
    xw = _windowize(x)                                     # [T,B,NW,WS,C]
    idx = _routing_idx(xw)                                 # [B,NW,TOPK]

    nc = _build_program(idx)

    # same-head block mask: mask[d, e] = (d//32 == e//32)
    r = np.arange(128) // 32
    mask = (r[:, None] == r[None, :]).astype(np.float32)

    in_maps = []
    for core in range(N_CORES):
        b, t = divmod(core, T)
        xwT_c = np.ascontiguousarray(
            xw[t, b].reshape(NTOK, C).T)                   # [C, NTOK]
        in_maps.append({
            "xwT": xwT_c,
            "masks": mask,
            "wq": np.asarray(W_qkv, np.float32),
            "bq": np.asarray(b_qkv, np.float32),
            "wp": np.asarray(W_proj, np.float32),
            "bp": np.asarray(b_proj, np.float32),
        })

    res = bass_utils.run_bass_kernel_spmd(
        nc, in_maps, core_ids=list(range(N_CORES)), trace=False)
    last_results = res
    last_nc, last_in_maps = nc, in_maps

    ow = np.empty((T, B, NW, WS, C), np.float32)
    for core in range(N_CORES):
        b, t = divmod(core, T)
        ow[t, b] = res.results[core]["out"].astype(np.float32).reshape(
            NW, WS, C)
    return _unwindowize(ow)
